# revision 49
# baseline (speedup 1.0000x reference)
"""Bass/Trainium2 kernel for nn_BoundaryLoss (8-core data-parallel).

loss = mean( ce * weight ) over (B=16, H=360, W=640) pixels, where
  ce     = logsumexp_c(pred) - pred[target]          (C=7)
  weight = 10 if 5x5-ellipse window around the pixel is NOT constant else 1
           (morphological gradient > 0, cv2 border-ignoring semantics)

Sharding: pure data parallel, 2 images per NeuronCore.  Each core emits two
[128, 32] f32 accumulator tiles holding per-partition partial sums
(w = 1 + 9*boundary):
  acc1 cols 0..11 : sum(w * lse)     per (group, half)   (Pool accum)
  acc2 cols 0..11 : sum(w * picked)  per (group, half)   (Pool accum)
Host: loss = ( S_wlse - S_wpk ) / (B*H*W)  -- the tiny 8-way combine is the
all-reduce from the sharding hint, done on host since kernel() returns the
full output anyway.

Morphology is computed exactly with the variance trick: the window is
constant  <=>  17*S2 == S1^2  where S1 = sum(t), S2 = sum(t^2) over the
17-tap ellipse with replicate clamping at borders.  All quantities are small
integers -> exact in fp16 matmuls + fp32 PSUM.

Structure: a 3-stage software pipeline over 6 row-groups --
load(g+2) / morph(g+1) / ce(g).  Engine assignment balances busy time:
Act: exp (fp8 out) + ln + S1^2;  DVE: masks, mask*P, t^2, weights;
Pool: DMA desc-gen + boundary mask + both weighted accumulations;
PE: morphology convs + fp8 DoubleRow channel-sum + picked-sum.
Morphology matmuls are 4 contiguous PSUM accumulation groups (S1 first --
it only needs t, not t^2) so the Tile scheduler cannot interleave them
with CE matmuls.  convw is split in 3 variant chunks; the later chunks
ride the Pool SWDGE queue behind the P loads so the serial DMA engine
resource services the critical first P transfer early.
"""

import sys

for _p in ("/opt/trn_rl_repo",):
    if _p not in sys.path:
        sys.path.insert(0, _p)

import numpy as np
import ml_dtypes

import bass_rust
import concourse.bass as bass
import concourse.mybir as mybir
from concourse.tile import TileContext
from concourse import bass_utils

F32 = mybir.dt.float32
F16 = mybir.dt.float16  # fp16: 10-bit mantissa, exact ints 0..2048
F8 = mybir.dt.float8e4  # e4m3
I32 = mybir.dt.int32

B_PER_CORE = 2
H, W, C = 360, 640, 7
# (row0, rows, variant): variant 0=top-clamped, 1=interior, 2=bottom-clamped
GROUPS = [(0, 124, 0), (124, 124, 1), (248, 112, 2)]
WPAD = W + 4
NCOL = 32  # acc tile columns

# convw slot layout: var0 dx0-4 -> slots 0-4, idw -> 5, var1 -> 6-10,
# var2 -> 11-15.  Chunk A = slots 0-5, chunk B = 6-10, chunk C = 11-15.
def _slot(var, dxi):
    return dxi if var == 0 else (1 + var * 5 + dxi)


IDW_SLOT = 5

# ellipse 5x5 taps grouped by dx -> vertical dy list
VERT = {0: [-2, -1, 0, 1, 2], -1: [-1, 0, 1], 1: [-1, 0, 1],
        -2: [-1, 0, 1], 2: [-1, 0, 1]}
DXS = [-2, -1, 0, 1, 2]


def _build_convw():
    """[128, 16*124] f16 banded vertical conv lhsT per (variant, dx) with
    border clamping baked in, plus an identity slot."""
    w = np.zeros((16, 128, 124), dtype=np.float32)
    seen = {}
    for (r0, R, v) in GROUPS:
        if v in seen:
            continue
        seen[v] = True
        in_r0 = max(r0 - 2, 0)
        in_r1 = min(r0 + R + 2, H)
        for dxi, dx in enumerate(DXS):
            for j in range(R):
                for dy in VERT[dx]:
                    rr = min(max(r0 + j + dy, 0), H - 1)
                    k = rr - in_r0
                    assert 0 <= k < in_r1 - in_r0 <= 128
                    w[_slot(v, dxi), k, j] += 1.0
    for k in range(124):
        w[IDW_SLOT, k, k] = 1.0
    return np.ascontiguousarray(
        w.transpose(1, 0, 2).reshape(128, 16 * 124)).astype(np.float16)


def _build_id8():
    """[128, 2, 128] fp8e4: identity in both DoubleRow k-tile slots."""
    eye = np.eye(128, dtype=np.float32)
    out = np.stack([eye, eye], axis=1)
    return np.ascontiguousarray(out).astype(ml_dtypes.float8_e4m3fn)


def split_multiwait_drains(nc, max_waits=1):
    """This walrus build rejects >1 sync-waits on CTRL-class instructions
    (the Tile end-of-kernel drain).  Split extra waits into preceding
    single-wait EventSemaphore instructions on the same engine."""
    fn = nc.m.functions[0]
    for bb in fn.blocks:
        for inst in list(bb.instructions):
            si = inst.sync_info
            if si is None or len(si.on_wait) <= max_waits:
                continue
            waits = list(si.on_wait)
            keep, extra = waits[:max_waits], waits[max_waits:]
            new_insts = []
            for k, wt in enumerate(extra):
                es = mybir.InstEventSemaphore(
                    name=f"{inst.name}-waitsplit-{k}", ins=[], outs=[])
                es.engine = inst.engine
                es.sync_info = bass_rust.SyncInfo(on_wait=[wt], on_update=[])
                nc.register_instruction(es, overwrite=True)
                new_insts.append(es)
            inst.sync_info = bass_rust.SyncInfo(
                on_wait=keep, on_update=list(si.on_update))
            pos = [i.name for i in bb.instructions].index(inst.name)
            for k, es in enumerate(new_insts):
                bb.instructions.insert(pos + k, es)


def _chain_dma_waits(nc, pairs):
    """Make each t-HWDGE DMA wait for an earlier P DMA's completion sem so
    instantly-ready HWDGE transfers don't jump the serial DMA-engine queue
    ahead of critical P loads."""
    if not pairs:
        return
    # cumulative per-sem update value in program order
    fn = nc.m.functions[0]
    cum = {}
    cum_at = {}
    for bb in fn.blocks:
        for inst in bb.instructions:
            si = inst.sync_info
            if si is None:
                continue
            for up in si.on_update:
                key = up.id
                cum[key] = cum.get(key, 0) + (up.update_value or 0)
                cum_at[(inst.name, key)] = cum[key]
    for t_bi, p_bi in pairs:
        t_inst = t_bi.ins if hasattr(t_bi, "ins") else t_bi
        p_inst = p_bi.ins if hasattr(p_bi, "ins") else p_bi
        psi = p_inst.sync_info
        if psi is None or not psi.on_update:
            continue
        up = psi.on_update[0]
        val = cum_at.get((p_inst.name, up.id))
        if not val:
            continue
        wt = bass_rust.SyncWait(sync_type=up.sync_type, id=up.id,
                                ant_name=up.ant_name,
                                wait_mode="sem-ge-imm", wait_value=val)
        tsi = t_inst.sync_info
        waits = list(tsi.on_wait) if tsi else []
        ups = list(tsi.on_update) if tsi else []
        waits.append(wt)
        t_inst.sync_info = bass_rust.SyncInfo(on_wait=waits, on_update=ups)


def _ginfo(gi):
    r0, R, var = GROUPS[gi]
    in_r0 = max(r0 - 2, 0)
    in_r1 = min(r0 + R + 2, H)
    return r0, R, var, in_r0, in_r1, in_r1 - in_r0


class Emitter:
    def __init__(self, nc, tc, pools, aps, s1sq_dve=True, m_eng="pool",
                 junk1_eng="pool", junk2_eng="pool", t2_eng="dve",
                 wt_eng="pool", exp_split=False, pads_pool=False):
        self.nc = nc
        self.s1sq_dve = s1sq_dve
        self.m_eng = m_eng
        self.junk1_eng = junk1_eng
        self.junk2_eng = junk2_eng
        self.t2_eng = t2_eng
        self.wt_eng = wt_eng
        self.pads_pool = pads_pool
        self.pk_first = False
        self.pk_full = False
        self.wl_dve = False
        self._pk = {}
        self.exp_split = exp_split
        self.tc = tc
        self.io, self.sm, self.ps, self.psm = pools
        (self.pred, self.target, self.convw_sb, self.id8_sb,
         self.acc1, self.acc2) = aps

    def load(self, b, gi, t_hwdge=False, p_split=False, t_ctr_hbm=False,
             p_first=False, ch_split=False):
        """DMA loads for (image b, row-group gi); returns the tiles.
        t_hwdge: fetch t as raw i32 over HWDGE (SP queue) + DVE convert --
        keeps the Pool engine free of t desc-gen.  The HWDGE t transfers
        are chained behind earlier P transfers post-build (see
        _delay_pairs) so they don't jump the serial DMA-engine queue."""
        nc, io, sm = self.nc, self.io, self.sm
        r0, R, var, in_r0, in_r1, n_in = _ginfo(gi)

        t_pad = sm.tile([128, WPAD], F16, tag="t_pad")
        self.last_t_inst = None

        def emit_t():
            if t_hwdge or getattr(self, 't_all_hwdge', False):
                t_i32 = sm.tile([128, W], I32, tag="t_i32")
                ti = nc.sync.dma_start(out=t_i32[:n_in, :],
                                       in_=self.target[b, in_r0:in_r1, :])
                nc.vector.tensor_copy(t_pad[:n_in, 2:2 + W], t_i32[:n_in, :])
                self.last_t_inst = ti
            else:
                nc.gpsimd.dma_start(out=t_pad[:n_in, 2:2 + W],
                                    in_=self.target[b, in_r0:in_r1, :])
        if not p_first:
            emit_t()
        # engines cannot address SBUF at a partition offset, so for groups
        # whose center rows sit at partition 2 of t_pad, peel the center
        # rows into their own tile with a cheap SBUF->SBUF HWDGE DMA (SP
        # queue: no Pool desc-gen cost, tiny wire cost)
        d0 = r0 - in_r0
        if d0 == 0:
            t_ctr = t_pad[:R, 2:2 + W]
        elif t_ctr_hbm:
            # fill-phase groups: fetch center rows straight from HBM on the
            # Pool SWDGE queue (converting) so MG isn't gated on the extra
            # post-t_pad HWDGE hop
            t_ctr_t = sm.tile([128, W], F16, tag="t_ctr")
            nc.gpsimd.dma_start(out=t_ctr_t[:R, :],
                                in_=self.target[b, r0:r0 + R, :])
            t_ctr = t_ctr_t[:R, :]
        else:
            t_ctr_t = sm.tile([128, W], F16, tag="t_ctr")
            nc.sync.dma_start(out=t_ctr_t[:R, :],
                              in_=t_pad[d0:d0 + R, 2:2 + W])
            t_ctr = t_ctr_t[:R, :]
        P = io.tile([128, C, W], F16, tag="P")
        self.last_p_inst = None
        if ch_split:
            for (c0, c1) in ((0, 4), (4, C)):
                self.last_p_inst = nc.gpsimd.dma_start(
                    out=P[:R, c0:c1, :],
                    in_=self.pred[b, c0:c1, r0:r0 + R, :]
                    .rearrange("c r w -> r c w"))
        elif p_split:
            for h in range(2):
                hs = slice(h * (W // 2), (h + 1) * (W // 2))
                nc.gpsimd.dma_start(
                    out=P[:R, :, hs],
                    in_=self.pred[b, :, r0:r0 + R, hs]
                    .rearrange("c r w -> r c w"))
        else:
            self.last_p_inst = nc.gpsimd.dma_start(
                out=P[:R, :, :],
                in_=self.pred[b, :, r0:r0 + R, :].rearrange("c r w -> r c w"))
        if p_first:
            emit_t()
        return t_pad, P, t_ctr

    def morph_conv(self, b, gi, t_pad, last=False):
        """Morphology conv part: pads, t^2, 4 contiguous conv groups."""
        nc, sm, psm = self.nc, self.sm, self.psm
        r0, R, var, in_r0, in_r1, n_in = _ginfo(gi)
        alu = mybir.AluOpType

        # horizontal replicate pad (2 cols each side)
        pad_e = nc.gpsimd if self.pads_pool else nc.vector
        pad_e.tensor_copy(t_pad[:n_in, 0:2],
                          t_pad[:n_in, 2:3].broadcast_to([n_in, 2]))
        pad_e.tensor_copy(t_pad[:n_in, W + 2:W + 4],
                          t_pad[:n_in, W + 1:W + 2].broadcast_to([n_in, 2]))
        t2_pad = sm.tile([128, WPAD], F16, tag="t2_pad")
        t2e = self.t2_eng
        if t2e == "act":
            nc.scalar.square(t2_pad[:n_in, :], t_pad[:n_in, :])
        else:
            eng = nc.vector if t2e == "dve" else nc.gpsimd
            eng.tensor_tensor(out=t2_pad[:n_in, :], in0=t_pad[:n_in, :],
                              in1=t_pad[:n_in, :], op=alu.mult)

        S1_ps = psm.tile([128, W], F32, tag="S1")
        S2_ps = psm.tile([128, W], F32, tag="S2")
        # 4 contiguous accumulation groups; S1 (from t) first so the PE can
        # start before t^2 is ready
        for dst, src in ((S1_ps, t_pad), (S2_ps, t2_pad)):
            for (c0, c1) in ((0, 512), (512, W)):
                for dxi, dx in enumerate(DXS):
                    co = _slot(var, dxi) * 124
                    nc.tensor.matmul(dst[:R, c0:c1],
                                     self.convw_sb[:n_in, co:co + R],
                                     src[:n_in, 2 + dx + c0:2 + dx + c1],
                                     start=(dxi == 0), stop=(dxi == 4))
        return S1_ps, S2_ps

    def morph_tail(self, b, gi, S1_ps, S2_ps):
        """Morphology tail: variance test -> Wt weight tile."""
        nc, sm = self.nc, self.sm
        r0, R, var, in_r0, in_r1, n_in = _ginfo(gi)
        alu = mybir.AluOpType

        S1sq = sm.tile([128, W], F32, tag="S1sq")
        if self.s1sq_dve:
            nc.vector.tensor_tensor(out=S1sq[:R, :], in0=S1_ps[:R, :],
                                    in1=S1_ps[:R, :], op=alu.mult)
        else:
            nc.scalar.square(S1sq[:R, :], S1_ps[:R, :])
        m = sm.tile([128, W], F16, tag="m")
        m_e = nc.gpsimd if self.m_eng == "pool" else nc.vector
        m_e.scalar_tensor_tensor(
            out=m[:R, :], in0=S2_ps[:R, :], scalar=17.0, in1=S1sq[:R, :],
            op0=alu.mult, op1=alu.is_gt)
        Wt = sm.tile([128, W], F16, tag="Wt")
        wt_e = nc.gpsimd if self.wt_eng == "pool" else nc.vector
        wt_e.tensor_scalar(out=Wt[:R, :], in0=m[:R, :], scalar1=9.0,
                           scalar2=1.0, op0=alu.mult, op1=alu.add)
        return Wt

    def ce(self, b, gi, t_pad, P, t_ctr, Wt, g, split=False, last=False,
           ch_split=False):
        """Cross-entropy terms + weighted accumulation for group g.
        split: emit exp/mask-mult per half so the tail chain of the last
        group starts as soon as half the data is through."""
        nc, io, sm, ps = self.nc, self.io, self.sm, self.ps
        r0, R, var, in_r0, in_r1, n_in = _ginfo(gi)
        d0 = r0 - in_r0
        alu = mybir.AluOpType
        AF = mybir.ActivationFunctionType

        MG = io.tile([128, C, W], F16, tag="MG")
        for c in range(C):
            nc.vector.tensor_scalar(out=MG[:R, c, :], in0=t_ctr,
                                    scalar1=float(c), scalar2=None,
                                    op0=alu.is_equal)
        E8 = io.tile([128, C, W], F8, tag="E8")
        MP = io.tile([128, C, W], F16, tag="MP")
        HW_ = W // 2
        split = split or self.exp_split
        if ch_split:
            nc.scalar.activation(E8[:R, 0:4, :], P[:R, 0:4, :], AF.Exp)
            nc.scalar.activation(E8[:R, 4:C, :], P[:R, 4:C, :], AF.Exp)
            nc.vector.tensor_tensor(out=MP[:R, :, :], in0=MG[:R, :, :],
                                    in1=P[:R, :, :], op=alu.mult)
        elif not split:
            nc.scalar.activation(E8[:R, :, :], P[:R, :, :], AF.Exp)
            nc.vector.tensor_tensor(out=MP[:R, :, :], in0=MG[:R, :, :],
                                    in1=P[:R, :, :], op=alu.mult)
        else:
            for h in range(2):
                hs = slice(h * HW_, (h + 1) * HW_)
                nc.scalar.activation(E8[:R, :, hs], P[:R, :, hs], AF.Exp)
                nc.vector.tensor_tensor(out=MP[:R, :, hs],
                                        in0=MG[:R, :, hs],
                                        in1=P[:R, :, hs], op=alu.mult)

        idw = self.convw_sb[:R, IDW_SLOT * 124:IDW_SLOT * 124 + R]
        id8 = self.id8_sb[:R, :, :R]

        if self.pk_first:
            if self.pk_full:
                PK_ps = ps.tile([128, W], F32, tag="PK")
                for (c0, c1) in ((0, 512), (512, W)):
                    for c in range(C):
                        nc.tensor.matmul(PK_ps[:R, c0:c1], idw,
                                         MP[:R, c, c0:c1],
                                         start=(c == 0), stop=(c == C - 1))
                self._pk[0] = PK_ps
            else:
                for h in range(2):
                    h0 = h * HW_
                    h1 = h0 + HW_
                    PK_ps = ps.tile([128, HW_], F32, tag="PK")
                    for c in range(C):
                        nc.tensor.matmul(PK_ps[:R, :], idw,
                                         MP[:R, c, h0:h1],
                                         start=(c == 0), stop=(c == C - 1))
                    self._pk[h] = PK_ps

        # channel-sum of E8 into a full-width 2-bank PSUM tile: 3 fp8
        # DoubleRow identity matmuls (2 ch each) + 1 plain fp8 identity
        # matmul for channel 6, split at the PSUM bank boundary
        S_ps = self.psm.tile([128, W], F32, tag="S")
        for (c0, c1) in ((0, 512), (512, W)):
            for ci in range(3):
                nc.tensor.matmul(S_ps[:R, c0:c1], id8,
                                 E8[:R, 2 * ci:2 * ci + 2, c0:c1],
                                 start=(ci == 0), stop=False,
                                 perf_mode=mybir.MatmulPerfMode.DoubleRow,
                                 skip_group_check=True)
            nc.tensor.matmul(S_ps[:R, c0:c1], self.id8_sb[:R, 0, :R],
                             E8[:R, 6, c0:c1], start=False, stop=True,
                             skip_group_check=True)
        lse = sm.tile([128, W], F16, tag="lse")
        nc.scalar.activation(lse[:R, :], S_ps[:R, :], AF.Ln)
        # weighted-lse accumulation: the product runs as a Pool TensorTensor
        # (one of the few ops the Q7 engine supports) and the free-dim
        # reduction as a DVE tensor_scalar accum (4x mode)
        if self.junk1_eng == "pool":
            WL = sm.tile([128, W], F16, tag="WL")
            wl_e = nc.vector if (last or self.wl_dve) else nc.gpsimd
            wl_e.tensor_tensor(out=WL[:R, :], in0=Wt[:R, :],
                               in1=lse[:R, :], op=alu.mult)
            junk1 = sm.tile([128, W], F16, tag="junk1")
            nc.vector.tensor_scalar(out=junk1[:R, :], in0=WL[:R, :],
                                    scalar1=1.0, scalar2=0.0, op0=alu.mult,
                                    op1=alu.add,
                                    accum_out=self.acc1[:, g:g + 1][:R, :])
        else:
            junk1 = sm.tile([128, W], F16, tag="junk1")
            nc.vector.scalar_tensor_tensor(
                out=junk1[:R, :], in0=Wt[:R, :], scalar=0.0,
                in1=lse[:R, :], op0=alu.bypass, op1=alu.mult,
                accum_out=self.acc1[:, g:g + 1][:R, :])

        if self.pk_full:
            PK_ps = self._pk[0]
            junk2 = sm.tile([128, W], F16, tag="junk2")
            nc.vector.scalar_tensor_tensor(
                out=junk2[:R, :], in0=Wt[:R, :], scalar=0.0,
                in1=PK_ps[:R, :], op0=alu.bypass, op1=alu.mult,
                accum_out=self.acc2[:, g:g + 1][:R, :])
        else:
            for h in range(2):
                h0 = h * HW_
                h1 = h0 + HW_
                if self.pk_first:
                    PK_ps = self._pk[h]
                else:
                    PK_ps = ps.tile([128, HW_], F32, tag="PK")
                    for c in range(C):
                        nc.tensor.matmul(PK_ps[:R, :], idw, MP[:R, c, h0:h1],
                                         start=(c == 0), stop=(c == C - 1))
                junk2 = sm.tile([128, HW_], F16, tag="junk2")
                nc.vector.scalar_tensor_tensor(
                    out=junk2[:R, :], in0=Wt[:R, h0:h1], scalar=0.0,
                    in1=PK_ps[:R, :], op0=alu.bypass, op1=alu.mult,
                    accum_out=self.acc2[:, 2 * g + h:2 * g + h + 1][:R, :])


def build_nc(io_bufs=3, sm_bufs=5, ps_bufs=2, pool_mode="stack",
             t0_hwdge=True, s1sq_dve=False, load_ahead=2, split_last=False,
             t_all_hwdge=False, split_first=False, m_eng="dve", junk1_eng="pool",
             junk2_eng="dve", t2_eng="pool", wt_eng="dve", exp_split=False,
             tail_after_ce=False, pads_pool=False, pk_first=True,
             tctr_hbm_max=5, p_first=False, wl_dve=False, ch_split0=False,
             chain_consts=False, small_first=False, pk_full=False):
    nc = bass.Bass()
    pred = nc.dram_tensor("pred", [B_PER_CORE, C, H, W], F32,
                          kind="ExternalInput")
    target = nc.dram_tensor("target", [B_PER_CORE, H, W], I32,
                            kind="ExternalInput")
    convw = nc.dram_tensor("convw", [128, 16 * 124], F16,
                           kind="ExternalInput")
    id8_in = nc.dram_tensor("id8", [128, 2, 128], F8, kind="ExternalInput")
    acc_out = nc.dram_tensor("acc", [128, 2 * NCOL], F32,
                             kind="ExternalOutput")

    CW = 124
    with TileContext(nc, pool_alloc_mode=pool_mode) as tc:
        with (
            tc.tile_pool(name="io", bufs=io_bufs) as io,
            tc.tile_pool(name="sm", bufs=sm_bufs) as sm,
            tc.tile_pool(name="ps", bufs=ps_bufs, space="PSUM") as ps,
            tc.tile_pool(name="psm", bufs=1, space="PSUM") as psm,
            tc.tile_pool(name="const", bufs=1) as cpool,
        ):
            convw_sb = cpool.tile([128, 16 * CW], F16)
            id8_sb = cpool.tile([128, 2, 128], F8)
            accT = cpool.tile([128, 2 * NCOL], F32)
            nc.vector.memset(accT[:, :], 0.0)
            acc1 = accT[:, 0:NCOL]
            acc2 = accT[:, NCOL:2 * NCOL]
            aps = (pred.ap(), target.ap(), convw_sb, id8_sb, acc1, acc2)
            em = Emitter(nc, tc, (io, sm, ps, psm), aps, s1sq_dve=s1sq_dve,
                         m_eng=m_eng, junk1_eng=junk1_eng,
                         junk2_eng=junk2_eng, t2_eng=t2_eng,
                         wt_eng=wt_eng, exp_split=exp_split,
                         pads_pool=pads_pool)
            em.pk_first = pk_first
            em.pk_full = pk_full
            em.wl_dve = wl_dve

            work = [(b, gi) for b in range(B_PER_CORE)
                    for gi in range(len(GROUPS))]
            if small_first:
                # lead with the 112-row group: its shorter P transfer
                # shortens the serial-DMA fill before the first exp
                work = [(0, 2), (0, 0), (0, 1), (1, 0), (1, 1), (1, 2)]
            n = len(work)
            em.t_all_hwdge = False
            delay_pairs0 = []
            prologue_p = {}
            tiles = {}
            tiles[0] = em.load(*work[0], t_hwdge=t0_hwdge,
                               p_split=split_first, p_first=p_first,
                               ch_split=ch_split0)
            prologue_p[0] = em.last_p_inst
            if p_first and t0_hwdge and em.last_t_inst is not None:
                delay_pairs0 = [(em.last_t_inst, em.last_p_inst)]
            else:
                delay_pairs0 = []
            # chunk A (var0 + idw) via HWDGE; chained behind t0 so its
            # transfer doesn't delay P(0) on the serial DMA-engine wire
            cwa_i = nc.sync.dma_start(out=convw_sb[:, :6 * CW],
                                      in_=convw.ap()[:, :6 * CW])
            id8_i = nc.sync.dma_start(out=id8_sb[:, :, :], in_=id8_in.ap())
            if chain_consts and em.last_t_inst is not None:
                delay_pairs0 += [(cwa_i, em.last_t_inst),
                                 (id8_i, em.last_t_inst)]
            # chunks B/C ride the Pool SWDGE queue behind the early loads;
            # each lands before its variant is needed by morph(1)/morph(2)
            nc.gpsimd.dma_start(out=convw_sb[:, 6 * CW:11 * CW],
                                in_=convw.ap()[:, 6 * CW:11 * CW])
            tiles[1] = em.load(*work[1], t_hwdge=t_all_hwdge,
                               t_ctr_hbm=True, p_first=p_first)
            prologue_p[1] = em.last_p_inst
            nc.gpsimd.dma_start(out=convw_sb[:, 11 * CW:],
                                in_=convw.ap()[:, 11 * CW:])
            wts = {}
            s12 = em.morph_conv(*work[0], tiles[0][0])
            wts[0] = em.morph_tail(*work[0], *s12)
            if load_ahead >= 3:
                tiles[2] = em.load(*work[2], t_hwdge=t_all_hwdge)
            p_insts = {}
            delay_pairs = []
            for g in range(n):
                if g + load_ahead < n and g + load_ahead >= 2:
                    gl = g + load_ahead
                    tiles[gl] = em.load(*work[gl], t_hwdge=t_all_hwdge,
                                        t_ctr_hbm=(gl <= tctr_hbm_max),
                                        p_first=p_first)
                    prev = p_insts.get(gl - 2) or prologue_p.get(gl - 2)
                    if em.last_t_inst is not None and prev is not None:
                        delay_pairs.append((em.last_t_inst, prev))
                    if em.last_p_inst is not None:
                        p_insts[gl] = em.last_p_inst
                if g + 1 < n:
                    s12 = em.morph_conv(*work[g + 1], tiles[g + 1][0])
                    if not tail_after_ce:
                        wts[g + 1] = em.morph_tail(*work[g + 1], *s12)
                em.ce(*work[g], tiles[g][0], tiles[g][1], tiles[g][2],
                      wts[g], g,
                      split=(split_last and g == n - 1) or
                            (split_first and g == 0),
                      last=(g == n - 1), ch_split=(ch_split0 and g == 0))
                if g + 1 < n and tail_after_ce:
                    wts[g + 1] = em.morph_tail(*work[g + 1], *s12)
                del tiles[g], wts[g]

            nc.sync.dma_start(out=acc_out.ap(), in_=accT[:, :])

    _chain_dma_waits(nc, delay_pairs + delay_pairs0)
    split_multiwait_drains(nc)
    return nc


_CACHED = {}


def _get_nc():
    if "nc" not in _CACHED:
        _CACHED["nc"] = build_nc()
        _CACHED["convw"] = _build_convw()
        _CACHED["id8"] = _build_id8()
    return _CACHED["nc"], _CACHED["convw"]


def combine_acc(results):
    """results: list of {'acc1': [128,32], 'acc2': [128,32]} -> scalar loss."""
    ng = B_PER_CORE * len(GROUPS)
    s_wlse = s_wpk = 0.0
    for r in results:
        a = r["acc"].astype(np.float64)
        s_wlse += a[:, 0:ng].sum()
        s_wpk += a[:, NCOL:NCOL + 2 * ng].sum()
    n = 16 * H * W
    loss = (s_wlse - s_wpk) / n
    return np.float32(loss)


def kernel(pred, target):
    nc, convw = _get_nc()
    id8 = _CACHED["id8"]
    n_cores = 8
    in_maps = []
    for i in range(n_cores):
        in_maps.append({
            "pred": np.ascontiguousarray(pred[2 * i:2 * i + 2]),
            "target": np.ascontiguousarray(target[2 * i:2 * i + 2]),
            "convw": convw,
            "id8": id8,
        })
    res = bass_utils.run_bass_kernel_spmd(nc, in_maps,
                                          core_ids=list(range(n_cores)))
    return combine_acc(res.results)
